# revision 1
# baseline (speedup 1.0000x reference)
"""Trainium2 Bass kernel: DiT block with cross-attention (nn_DiTBlock_CrossAttn).

Sharding: pure data-parallel over batch. B=8 batch elements -> 8 NeuronCores,
one batch element per core, no collectives. Each core runs the full block:
adaLN -> self-attn -> cross-attn -> FFN (exact GELU).

Layout: activations kept feature-major ("transposed", [feature_part, chunk, token])
so every projection is matmul(lhsT=W[din,dout], rhs=actT[din,n]) with weights in
their natural DRAM layout. Attention uses the S^T orientation with a fused
ones-column in V for the softmax denominator (softmax without max subtraction is
safe: |logits| < ~5 for this problem family). Matmuls run in bf16 (weights are
pre-cast on host), accumulation and residual stream stay fp32.
"""
import contextlib

import numpy as np
import ml_dtypes

import concourse.bass as bass
import concourse.tile as tile
import concourse.mybir as mybir
from concourse import bacc
from concourse.bass_utils import run_bass_kernel_spmd
from concourse.masks import make_identity

P = 128
N = 1024            # tokens
D = 1024            # hidden
KD = D // P         # 8 feature chunks of hidden
NT = N // P         # 8 token tiles
H = 16              # heads
HD = 64             # head dim
S = 256             # context tokens
ST = S // P         # 2
CD = 512            # context dim
CKD = CD // P       # 4
MLP = 4096
MT = MLP // P       # 32
EPS = 1e-6
ASCALE = 0.125      # 1/sqrt(HD)
NCORES = 8

F32 = mybir.dt.float32
BF16 = mybir.dt.bfloat16
AF = mybir.ActivationFunctionType
OP = mybir.AluOpType


def _wcols(w):
    """[din, dout] dram AP -> [p, ko, dout] (feature-chunked lhsT view)."""
    return w.rearrange("(ko p) f -> p ko f", p=P)


def build_nc(taps=(), upto='full'):
    nc = bacc.Bacc("TRN2", target_bir_lowering=False, debug=False)

    d = {}
    d['x'] = nc.dram_tensor("x", [N, D], F32, kind="ExternalInput").ap()
    d['c'] = nc.dram_tensor("c", [D], F32, kind="ExternalInput").ap()
    d['context'] = nc.dram_tensor("context", [S, CD], F32, kind="ExternalInput").ap()
    for nm, sh in [("w_qkv", [D, 3 * D]), ("w_so", [D, D]), ("w_cq", [D, D]),
                   ("w_ck", [CD, D]), ("w_cv", [CD, D]), ("w_co", [D, D]),
                   ("w1", [D, MLP]), ("w2", [MLP, D]), ("w_ada", [D, 6 * D])]:
        d[nm] = nc.dram_tensor(nm, sh, BF16, kind="ExternalInput").ap()
    for nm, sh in [("b_qkv", [3 * D]), ("b_so", [D]), ("b_cq", [D]), ("b_ck", [D]),
                   ("b_cv", [D]), ("b_co", [D]), ("b1", [MLP]), ("b2", [D]),
                   ("b_ada", [6 * D])]:
        d[nm] = nc.dram_tensor(nm, sh, F32, kind="ExternalInput").ap()
    out = nc.dram_tensor("out_x", [N, D], F32, kind="ExternalOutput").ap()
    srows = nc.dram_tensor("srows", [40, N], F32).ap()
    g_dram = nc.dram_tensor("g_dram", [MT, P, N], BF16).ap()

    tap_shapes = {
        "ada": ([P, 48], F32), "h1": ([P, KD, N], BF16),
        "q": ([P, KD, N], BF16), "k": ([P, KD, N], BF16),
        "v65": ([P, NT, H, 65], BF16), "saO": ([P, KD, N], BF16),
        "x2": ([P, KD, N], F32), "h2": ([P, KD, N], BF16),
        "cq": ([P, KD, N], BF16), "ck": ([P, KD, S], BF16),
        "cv65": ([P, ST, H, 65], BF16), "caO": ([P, KD, N], BF16),
        "x3": ([P, KD, N], F32), "h3": ([P, KD, N], BF16),
        "xT": ([P, KD, N], F32),
    }
    tap_aps = {nm: nc.dram_tensor(f"dbg_{nm}", *tap_shapes[nm], kind="ExternalOutput").ap()
               for nm in taps}

    with tile.TileContext(nc) as tc:
        _emit(nc, tc, d, out, srows, g_dram, tap_aps, upto)
    nc.compile()
    return nc


def _emit(nc, tc, d, out, srows, g_dram, tap_aps={}, upto='full'):
    def tap(nm, t):
        if nm in tap_aps:
            nc.sync.dma_start(tap_aps[nm], t[:])

    gl = contextlib.ExitStack()          # global pools, whole-kernel lifetime
    with gl:
        const = gl.enter_context(tc.tile_pool(name="const", bufs=1))
        resid = gl.enter_context(tc.tile_pool(name="resid", bufs=2))
        wpool = gl.enter_context(tc.tile_pool(name="wpool", bufs=3))
        bigbf = gl.enter_context(tc.tile_pool(name="bigbf", bufs=3))

        # ---------- constants ----------
        ident = const.tile([P, P], F32, tag="ident")
        make_identity(nc, ident)
        onesD_mat = const.tile([P, P], BF16, tag="onesD_mat")
        nc.vector.memset(onesD_mat[:], 1.0 / D)
        onesD_row = const.tile([P, 1], BF16, tag="onesD_row")
        nc.vector.memset(onesD_row[:], 1.0 / D)
        ones1_f = const.tile([1, P], F32, tag="ones1_f")
        nc.vector.memset(ones1_f[:], 1.0)
        eps_t = const.tile([P, 1], F32, tag="eps")
        nc.vector.memset(eps_t[:], EPS)

        ctxT = const.tile([P, CKD, S], BF16, tag="ctxT")
        ada = const.tile([P, 48], F32, tag="ada")
        splus = const.tile([P, 24], F32, tag="splus")
        xT = resid.tile([P, KD, N], F32, tag="resid")

        def partial_out(ref_tile):
            for k in range(KD):
                nc.sync.dma_start(out[k * P:(k + 1) * P, :], ref_tile[:, k, :])

        # ---------- staging scope ----------
        st = contextlib.ExitStack()
        stg = st.enter_context(tc.tile_pool(name="stg", bufs=4))
        ps_t = st.enter_context(tc.tile_pool(name="ps_t", bufs=3, space="PSUM"))

        def bias_T(name, brow, width):
            stage = stg.tile([width, P], F32, tag="btmp")
            nc.sync.dma_start(stage[:], brow.rearrange("(a p) -> a p", p=P))
            ps = ps_t.tile([P, 512], F32, tag="pst")
            nc.tensor.transpose(ps[:, 0:width], stage[:], ident[0:width, 0:width])
            t = const.tile([P, width], F32, tag=f"bT_{name}")
            nc.vector.tensor_copy(t[:], ps[:, 0:width])
            return t

        b_qkvT = bias_T("qkv", d['b_qkv'], 24)
        b_soT = bias_T("so", d['b_so'], KD)
        b_cqT = bias_T("cq", d['b_cq'], KD)
        b_ckT = bias_T("ck", d['b_ck'], KD)
        b_coT = bias_T("co", d['b_co'], KD)
        b1T = bias_T("b1", d['b1'], MT)
        b_adaT = bias_T("ada", d['b_ada'], 48)
        b2T = bias_T("b2", d['b2'], KD)

        # x -> xT (feature-major, fp32, via PE transpose)
        for i in range(NT):
            xs = stg.tile([P, D], F32, tag="xstage")
            nc.sync.dma_start(xs[:], d['x'][i * P:(i + 1) * P, :])
            for jg in range(2):
                ps = ps_t.tile([P, 512], F32, tag="pst")
                for j4 in range(4):
                    j = jg * 4 + j4
                    nc.tensor.transpose(ps[:, j4 * 128:(j4 + 1) * 128],
                                        xs[:, j * 128:(j + 1) * 128], ident[:])
                nc.vector.tensor_copy(
                    xT[:, jg * 4:(jg + 1) * 4, i * P:(i + 1) * P],
                    ps.rearrange("p (a b) -> p a b", a=4))

        # context -> ctxT (bf16)
        for i in range(ST):
            cs = stg.tile([P, D], F32, tag="xstage")
            nc.sync.dma_start(cs[:, 0:CD], d['context'][i * P:(i + 1) * P, :])
            ps = ps_t.tile([P, 512], F32, tag="pst")
            for j in range(4):
                nc.tensor.transpose(ps[:, j * 128:(j + 1) * 128],
                                    cs[:, j * 128:(j + 1) * 128], ident[:])
            nc.vector.tensor_copy(
                ctxT[:, :, i * P:(i + 1) * P],
                ps.rearrange("p (a b) -> p a b", a=4))

        # c -> silu(c)^T (bf16, feature-major [P, KD])
        cst = stg.tile([KD, P], F32, tag="cstage")
        nc.sync.dma_start(cst[:], d['c'].rearrange("(a p) -> a p", p=P))
        csil = stg.tile([KD, P], F32, tag="cstage")
        nc.scalar.activation(csil[:], cst[:], AF.Silu)
        pcs = ps_t.tile([P, 512], F32, tag="pst")
        nc.tensor.transpose(pcs[:, 0:KD], csil[:], ident[0:KD, 0:KD])
        silu_cT = const.tile([P, KD], BF16, tag="silu_cT")
        nc.vector.tensor_copy(silu_cT[:], pcs[:, 0:KD])

        # ada = silu(c) @ w_ada + b_ada  -> feature-major [P, 48]
        wada = _wcols(d['w_ada'])
        ada_blocks = 0 if upto == 'stage_noada' else 6
        if upto == 'stage_dmaonly':
            # DMA w_ada blocks but skip the matmuls; consume via tiny copy
            for blk in range(6):
                wb = wpool.tile([P, KD, 1024], BF16, tag="wblk")
                nc.sync.dma_start(wb[:], wada[:, :, blk * 1024:(blk + 1) * 1024])
                nc.vector.tensor_copy(ada[:, blk:blk+1].bitcast(BF16)[:, 0:1], wb[:, 0, 0:1])
            ada_blocks = 0
        if ada_blocks == 0:
            nc.vector.memset(ada[:], 0.01)
        for blk in range(ada_blocks):
            wb = wpool.tile([P, KD, 1024], BF16, tag="wblk")
            nc.sync.dma_start(wb[:], wada[:, :, blk * 1024:(blk + 1) * 1024])
            for t8 in range(8):
                t = blk * 8 + t8
                ps = ps_t.tile([P, 512], F32, tag="pst")
                for k in range(KD):
                    nc.tensor.matmul(ps[:, 0:1], wb[:, k, t8 * 128:(t8 + 1) * 128],
                                     silu_cT[:, k:k + 1],
                                     start=(k == 0), stop=(k == KD - 1))
                nc.vector.tensor_copy(ada[:, t:t + 1], ps[:, 0:1])
        nc.vector.tensor_add(ada[:], ada[:], b_adaT[:])
        for g in range(3):
            nc.vector.tensor_scalar_add(splus[:, g * 8:(g + 1) * 8],
                                        ada[:, g * 16 + 8:g * 16 + 16], 1.0)
        tap("ada", ada)
        tap("xT", xT)
        st.close()

        if upto in ('stage', 'stage_noada', 'stage_dmaonly'):
            partial_out(xT)
            return
        # ---------- LN + modulate (self-contained pool scope) ----------
        def ln_mod(x_in, g):
            h_out = bigbf.tile([P, KD, N], BF16, tag="big")
            ls = contextlib.ExitStack()
            with ls:
                lnb = ls.enter_context(tc.tile_pool(name="lnb", bufs=2))
                lrows = ls.enter_context(tc.tile_pool(name="lrows", bufs=3))
                ps_mu = ls.enter_context(tc.tile_pool(name="ps_mu", bufs=1, space="PSUM"))
                ps_rs = ls.enter_context(tc.tile_pool(name="ps_rs", bufs=1, space="PSUM"))
                ps_e2 = ls.enter_context(tc.tile_pool(name="ps_e2", bufs=1, space="PSUM"))
                # mean lands ALREADY BROADCAST across partitions: lhsT is the
                # all-ones(1/D) matrix, so every psum partition gets mean(x).
                mu_ps = ps_mu.tile([P, N], F32, tag="mups")
                e2_ps = ps_e2.tile([1, N], F32, tag="e2ps")
                for k in range(KD):
                    xbf = lnb.tile([P, N], BF16, tag="lnxbf")
                    nc.vector.tensor_copy(xbf[:], x_in[:, k])
                    sq = lnb.tile([P, N], BF16, tag="lnsq")
                    nc.scalar.activation(sq[:], xbf[:], AF.Square)
                    for half in range(2):
                        hs = slice(half * 512, (half + 1) * 512)
                        nc.tensor.matmul(mu_ps[:, hs], onesD_mat[:], xbf[:, hs],
                                         start=(k == 0), stop=(k == KD - 1))
                        nc.tensor.matmul(e2_ps[:, hs], onesD_row[:], sq[:, hs],
                                         start=(k == 0), stop=(k == KD - 1))
                murow = lrows.tile([1, N], F32, tag="row")
                nc.vector.tensor_copy(murow[:], mu_ps[0:1, :])
                var = lrows.tile([1, N], F32, tag="row")
                nc.vector.tensor_mul(var[:], murow[:], murow[:])
                e2row = lrows.tile([1, N], F32, tag="row")
                nc.vector.tensor_copy(e2row[:], e2_ps[:])
                nc.vector.tensor_sub(var[:], e2row[:], var[:])
                nc.scalar.activation(var[:], var[:], AF.Sqrt, bias=eps_t[0:1])
                nc.vector.reciprocal(var[:], var[:])
                # broadcast rstd across partitions with a K=1 ones matmul
                rs_ps = ps_rs.tile([P, N], F32, tag="rsps")
                for half in range(2):
                    hs = slice(half * 512, (half + 1) * 512)
                    nc.tensor.matmul(rs_ps[:, hs], ones1_f[:], var[:, hs],
                                     start=True, stop=True)
                for k in range(KD):
                    t1 = lnb.tile([P, N], F32, tag="lnt1")
                    nc.vector.tensor_sub(t1[:], x_in[:, k], mu_ps[:])
                    nc.vector.tensor_mul(t1[:], t1[:], rs_ps[:])
                    nc.gpsimd.tensor_scalar(h_out[:, k], t1[:],
                                            splus[:, g * 8 + k:g * 8 + k + 1],
                                            ada[:, g * 16 + k:g * 16 + k + 1],
                                            OP.mult, OP.add)
            return h_out

        # ---------- generic transposed projection (512-wide weight blocks) ----
        def proj_T(ps_mm, w_cols, kdin, act_bf, n_free, dout, evict):
            nhalf = max(1, n_free // 512)
            for blk in range(dout // 1024):
                wb = wpool.tile([P, kdin, 1024], BF16, tag="wblk")
                nc.sync.dma_start(wb[:], w_cols[:, :, blk * 1024:(blk + 1) * 1024])
                for t8 in range(8):
                    ps = ps_mm.tile([P, N], F32, tag="pmm")
                    for half in range(nhalf):
                        hs = slice(half * 512, half * 512 + min(512, n_free))
                        for k in range(kdin):
                            nc.tensor.matmul(ps[:, hs],
                                             wb[:, k, t8 * 128:(t8 + 1) * 128],
                                             act_bf[:, k, hs],
                                             start=(k == 0), stop=(k == kdin - 1))
                    evict(blk * 8 + t8, ps)

        # ---------- token-major V projection (fused ones column) ----------
        def proj_V(ps_mm, w_cols, kdin, act_bf, m_tiles, v65, bias_b):
            wb = wpool.tile([P, kdin, 1024], BF16, tag="wblk")
            nc.sync.dma_start(wb[:], w_cols[:])
            for blk in range(2):
                for i in range(m_tiles):
                    ps = ps_mm.tile([P, N], F32, tag="pmm")
                    for k in range(kdin):
                        nc.tensor.matmul(ps[:, 0:512],
                                         act_bf[:, k, i * 128:(i + 1) * 128],
                                         wb[:, k, blk * 512:(blk + 1) * 512],
                                         start=(k == 0), stop=(k == kdin - 1))
                    nc.vector.tensor_add(
                        v65[:, i, blk * 8:(blk + 1) * 8, 0:64],
                        ps[:, 0:512].rearrange("p (h e) -> p h e", h=8),
                        bias_b[:, blk * 512:(blk + 1) * 512]
                        .rearrange("p (h e) -> p h e", h=8))
            nc.vector.memset(v65[:, :, :, 64:65], 1.0)

        # ---------- attention core (self-contained pool scope) ----------
        def attention(q_T, kv_T, v65, m_tiles, o_bf, srow_base):
            at = contextlib.ExitStack()
            with at:
                expp = at.enter_context(tc.tile_pool(name="expp", bufs=6))
                arows = at.enter_context(tc.tile_pool(name="arows", bufs=2))
                rb = at.enter_context(tc.tile_pool(name="rb", bufs=2))
                ps_lg = at.enter_context(tc.tile_pool(name="ps_lg", bufs=2, space="PSUM"))
                ps_pv = at.enter_context(tc.tile_pool(name="ps_pv", bufs=3, space="PSUM"))
                for h in range(H):
                    pr, off = h // 2, (h % 2) * 64
                    pv = ps_pv.tile([65, N], F32, tag="pv")
                    # software pipeline over (mt, half) items: logits+exp run two
                    # items ahead of the PV accumulation so the PE never waits on
                    # the ACT exp eviction.
                    items = m_tiles * 2
                    exs = [None] * items
                    def lgexp(i):
                        mt, half = i // 2, i % 2
                        hs = slice(half * 512, (half + 1) * 512)
                        lg = ps_lg.tile([P, 512], F32, tag="lg", name=f"lg{h}_{i}")
                        nc.tensor.matmul(
                            lg[:],
                            kv_T[off:off + 64, pr, mt * 128:(mt + 1) * 128],
                            q_T[off:off + 64, pr, hs],
                            start=True, stop=True)
                        ex = expp.tile([P, 512], BF16, tag="expT", name=f"ex{h}_{i}")
                        nc.scalar.activation(ex[:], lg[:], AF.Exp, scale=ASCALE)
                        exs[i] = ex
                    def pvacc(i):
                        mt, half = i // 2, i % 2
                        hs = slice(half * 512, (half + 1) * 512)
                        nc.tensor.matmul(pv[:, hs], v65[:, mt, h, :], exs[i][:],
                                         start=(mt == 0), stop=(mt == m_tiles - 1))
                    for i in range(items + 2):
                        if i < items:
                            lgexp(i)
                        if i >= 2:
                            pvacc(i - 2)
                    rec = arows.tile([1, N], F32, tag="row")
                    nc.vector.reciprocal(rec[:], pv[64:65, :])
                    r = srow_base + h
                    nc.sync.dma_start(srows[r:r + 1, :], rec[:])
                    rbt = rb.tile([64, N], F32, tag="rbt")
                    nc.sync.dma_start(rbt[:], srows[r:r + 1, :].partition_broadcast(64))
                    nc.vector.tensor_mul(o_bf[off:off + 64, pr, :], pv[0:64, :], rbt[:])

        # ================= self-attention =================
        h1 = ln_mod(xT, 0)

        qT = bigbf.tile([P, KD, N], BF16, tag="big")
        kT = bigbf.tile([P, KD, N], BF16, tag="big")

        sa_es = contextlib.ExitStack()
        vp = sa_es.enter_context(tc.tile_pool(name="vp", bufs=1))
        vb = sa_es.enter_context(tc.tile_pool(name="vb", bufs=1))
        v65 = vp.tile([P, NT, H, 65], BF16, tag="v65")
        vbias_b = vb.tile([P, D], F32, tag="vbias")
        nc.sync.dma_start(vbias_b[:],
                            d['b_qkv'][2 * D:3 * D][None, :].partition_broadcast(P))

        qkv_ps = contextlib.ExitStack()
        ps_mm = qkv_ps.enter_context(tc.tile_pool(name="ps_mm", bufs=2, space="PSUM"))

        def ev_qk(t, ps):
            dst = qT if t < 8 else kT
            nc.scalar.activation(dst[:, t % 8, :], ps[:], AF.Identity,
                                 bias=b_qkvT[:, t:t + 1])
        proj_V(ps_mm, _wcols(d['w_qkv'])[:, :, 2 * D:3 * D], KD, h1, NT, v65, vbias_b)
        proj_T(ps_mm, _wcols(d['w_qkv'])[:, :, 0:2 * D], KD, h1, N, 2 * D, ev_qk)
        tap("h1", h1); tap("q", qT); tap("k", kT); tap("v65", v65)
        qkv_ps.close()

        if upto == 'qkv':
            sa_es.close()
            partial_out(xT)
            return
        saO = bigbf.tile([P, KD, N], BF16, tag="big")
        attention(qT, kT, v65, NT, saO, 6)
        tap("saO", saO)
        sa_es.close()
        if upto == 'sa':
            partial_out(xT)
            return

        x2T = resid.tile([P, KD, N], F32, tag="resid")
        so_ps = contextlib.ExitStack()
        ps_mm = so_ps.enter_context(tc.tile_pool(name="ps_mm", bufs=2, space="PSUM"))

        def ev_so(t, ps):
            nc.vector.tensor_scalar_add(x2T[:, t, :], ps[:], b_soT[:, t:t + 1])
            nc.vector.tensor_add(x2T[:, t, :], x2T[:, t, :], xT[:, t, :])
        proj_T(ps_mm, _wcols(d['w_so']), KD, saO, N, D, ev_so)
        tap("x2", x2T)
        so_ps.close()

        # ================= cross-attention =================
        # ck/cv depend only on ctxT -> emit BEFORE LN2 so the PE has work
        # during the LN serial chain (own small psum pool; 2 banks).
        ca_es = contextlib.ExitStack()
        kp = ca_es.enter_context(tc.tile_pool(name="kp", bufs=1))
        vp = ca_es.enter_context(tc.tile_pool(name="vp2", bufs=1))
        vb = ca_es.enter_context(tc.tile_pool(name="vb2", bufs=1))
        ckT = kp.tile([P, KD, S], BF16, tag="ckT")
        cv65 = vp.tile([P, ST, H, 65], BF16, tag="cv65")
        cvbias_b = vb.tile([P, D], F32, tag="cvbias")
        nc.sync.dma_start(cvbias_b[:], d['b_cv'][None, :].partition_broadcast(P))

        ckcv_ps = contextlib.ExitStack()
        ps_kv = ckcv_ps.enter_context(tc.tile_pool(name="ps_kv", bufs=1, space="PSUM"))

        def ev_ck(t, ps):
            nc.scalar.activation(ckT[:, t, :], ps[:, 0:S], AF.Identity,
                                 bias=b_ckT[:, t:t + 1])
        proj_T(ps_kv, _wcols(d['w_ck']), CKD, ctxT, S, D, ev_ck)
        proj_V(ps_kv, _wcols(d['w_cv']), CKD, ctxT, ST, cv65, cvbias_b)
        tap("ck", ckT); tap("cv65", cv65)

        h2 = ln_mod(x2T, 1)
        ckcv_ps.close()

        cqT = bigbf.tile([P, KD, N], BF16, tag="big")
        ca_ps = contextlib.ExitStack()
        ps_mm = ca_ps.enter_context(tc.tile_pool(name="ps_mm", bufs=2, space="PSUM"))

        def ev_cq(t, ps):
            nc.scalar.activation(cqT[:, t, :], ps[:], AF.Identity,
                                 bias=b_cqT[:, t:t + 1])
        proj_T(ps_mm, _wcols(d['w_cq']), KD, h2, N, D, ev_cq)
        tap("h2", h2); tap("cq", cqT)
        ca_ps.close()

        caO = bigbf.tile([P, KD, N], BF16, tag="big")
        attention(cqT, ckT, cv65, ST, caO, 22)
        tap("caO", caO)
        ca_es.close()

        x3T = resid.tile([P, KD, N], F32, tag="resid")
        co_ps = contextlib.ExitStack()
        ps_mm = co_ps.enter_context(tc.tile_pool(name="ps_mm", bufs=2, space="PSUM"))

        def ev_co(t, ps):
            nc.vector.tensor_scalar_add(x3T[:, t, :], ps[:], b_coT[:, t:t + 1])
            nc.vector.tensor_add(x3T[:, t, :], x3T[:, t, :], x2T[:, t, :])
        proj_T(ps_mm, _wcols(d['w_co']), KD, caO, N, D, ev_co)
        tap("x3", x3T)
        co_ps.close()

        if upto == 'ca':
            partial_out(x3T)
            return
        # ================= FFN =================
        h3 = ln_mod(x3T, 2)
        # fold b2 into the residual before the final transpose-accumulate
        for k in range(KD):
            nc.vector.tensor_scalar_add(x3T[:, k, :], x3T[:, k, :], b2T[:, k:k + 1])

        w1_es = contextlib.ExitStack()
        gstage = w1_es.enter_context(tc.tile_pool(name="gstage", bufs=3))
        ps_mm = w1_es.enter_context(tc.tile_pool(name="ps_mm", bufs=2, space="PSUM"))

        def ev_g(t, ps):
            gst = gstage.tile([P, N], BF16, tag="gst")
            nc.scalar.activation(gst[:], ps[:], AF.Gelu, bias=b1T[:, t:t + 1])
            nc.sync.dma_start(g_dram[t], gst[:])
        proj_T(ps_mm, _wcols(d['w1']), KD, h3, N, MLP, ev_g)
        tap("h3", h3)
        w1_es.close()

        if upto == 'w1':
            partial_out(x3T)
            return
        w2_es = contextlib.ExitStack()
        ghp = w2_es.enter_context(tc.tile_pool(name="ghp", bufs=1))
        outst = w2_es.enter_context(tc.tile_pool(name="outst", bufs=3))
        ps_tt = w2_es.enter_context(tc.tile_pool(name="ps_tt", bufs=2, space="PSUM"))
        ps_po = w2_es.enter_context(tc.tile_pool(name="ps_po", bufs=4, space="PSUM"))

        # token-major copy of x3 (+b2), built via PE transposes into bigbf slots
        xtok = [bigbf.tile([P, 4, D], F32, tag="big", name=f"xtok{_i}")
                for _i in range(2)]
        for i in range(NT):
            dst = xtok[i // 4]
            for jg in range(2):
                ps = ps_tt.tile([P, 512], F32, tag="ptt")
                for j4 in range(4):
                    j = jg * 4 + j4
                    nc.tensor.transpose(ps[:, j4 * 128:(j4 + 1) * 128],
                                        x3T[:, j, i * P:(i + 1) * P], ident[:])
                nc.vector.tensor_copy(dst[:, i % 4, jg * 512:(jg + 1) * 512], ps[:])

        w2cols = d['w2'].rearrange("(ko p) f -> p ko f", p=P)
        for nh in range(2):
            gh = ghp.tile([P, MT, 512], BF16, tag="gh")
            for k in range(MT):
                nc.sync.dma_start(gh[:, k, :], g_dram[k, :, nh * 512:(nh + 1) * 512])
            for dq in range(4):
                w2q = wpool.tile([P, MT, 256], BF16, tag="wblk")
                nc.sync.dma_start(w2q[:], w2cols[:, :, dq * 256:(dq + 1) * 256])
                for i4 in range(4):
                    i = nh * 4 + i4
                    po = ps_po.tile([P, 256], F32, tag="po")
                    for k in range(MT):
                        nc.tensor.matmul(po[:], gh[:, k, i4 * 128:(i4 + 1) * 128],
                                         w2q[:, k, :],
                                         start=(k == 0), stop=(k == MT - 1))
                    ost = outst.tile([P, 256], F32, tag="ost")
                    nc.vector.tensor_add(
                        ost[:], po[:],
                        xtok[i // 4][:, i % 4, dq * 256:(dq + 1) * 256])
                    nc.sync.dma_start(out[i * P:(i + 1) * P, dq * 256:(dq + 1) * 256],
                                      ost[:])
        w2_es.close()


_NC = None


def _get_nc():
    global _NC
    if _NC is None:
        _NC = build_nc()
    return _NC


def make_in_maps(inputs):
    wnames = ["w_qkv", "w_so", "w_cq", "w_ck", "w_cv", "w_co", "w1", "w2", "w_ada"]
    bnames = ["b_qkv", "b_so", "b_cq", "b_ck", "b_cv", "b_co", "b1", "b2", "b_ada"]
    shared = {}
    for nm in wnames:
        shared[nm] = np.ascontiguousarray(
            np.asarray(inputs[nm]).astype(ml_dtypes.bfloat16))
    for nm in bnames:
        shared[nm] = np.ascontiguousarray(np.asarray(inputs[nm], dtype=np.float32))
    x = np.asarray(inputs['x'], dtype=np.float32)
    c = np.asarray(inputs['c'], dtype=np.float32)
    ctxt = np.asarray(inputs['context'], dtype=np.float32)
    in_maps = []
    for i in range(NCORES):
        m = dict(shared)
        m['x'] = np.ascontiguousarray(x[i])
        m['c'] = np.ascontiguousarray(c[i])
        m['context'] = np.ascontiguousarray(ctxt[i])
        in_maps.append(m)
    return in_maps


def kernel(**inputs):
    nc = _get_nc()
    in_maps = make_in_maps(inputs)
    res = run_bass_kernel_spmd(nc, in_maps, core_ids=list(range(NCORES)))
    return np.stack([res.results[i]["out_x"] for i in range(NCORES)]).astype(np.float32)


if __name__ == "__main__":
    data = np.load("/root/problem/inputs.npz")
    out = kernel(**{k: data[k] for k in data.files})
    gold = np.load("/root/problem/gold64.npy")
    err = np.abs(out - gold)
    print("max abs err:", err.max(), " rel:", err.max() / np.abs(gold).max())



# revision 11
# speedup vs baseline: 1.1694x; 1.1694x over previous
"""Trainium2 Bass kernel: DiT block with cross-attention (nn_DiTBlock_CrossAttn).

Sharding: pure data-parallel over batch. B=8 -> 8 NeuronCores, no collectives.

v2 design vs baseline:
- Host feeds x/context/biases pre-transposed (feature-major); output is written
  feature-major and transposed back on host. No PE transposes of activations.
- All six attention-side projections (qkv, so, cq, ck, cv, co) run in fp8 e4m3
  with DoubleRow perf mode: one matmul contracts 256 rows (two 128-chunks), so
  instruction count and PE streaming time halve. Weights are host-scaled x32;
  the 1/32 dequant is folded into eviction scale/bias ops. FFN and attention
  internals stay bf16 (fp8 there breaks the 2e-2 accuracy gate).
- adaLN restructured: silu(c) chunks are the stationary operand (1-column
  ldweights) with w_ada streamed as the moving operand: 96 N=512 matmuls
  instead of 384 tiny ones.
- Attention processes head PAIRS: the two K=64 logit matmuls of heads 2i/2i+1
  target row-groups 0-63/64-127 and run concurrently on the PE array.
- FFN: gelu activations stay in SBUF (no DRAM roundtrip); w2 runs feature-major
  so the residual add needs no transposes.
- LayerNorm mean/E[x2] share one all-ones stationary operand; rstd is computed
  broadcast so no extra broadcast matmuls are needed.
- Residual stream in bf16.
"""
import contextlib

import numpy as np
import ml_dtypes

import concourse.bass as bass
import concourse.tile as tile
import concourse.mybir as mybir
from concourse import bacc
from concourse.bass_utils import run_bass_kernel_spmd
from concourse.masks import make_identity

P = 128
N = 1024            # tokens
D = 1024            # hidden
KD = D // P         # 8 feature chunks
NT = N // P         # 8 token tiles
H = 16              # heads
HD = 64             # head dim
S = 256             # context tokens
ST = S // P         # 2
CD = 512            # context dim
CKD = CD // P       # 4
MLP = 4096
MT = MLP // P       # 32
EPS = 1e-6
ASCALE = 0.125      # 1/sqrt(HD)
NCORES = 8
WS = 32.0           # fp8 weight pre-scale (host side)
IWS = 1.0 / WS

F32 = mybir.dt.float32
BF16 = mybir.dt.bfloat16
F8 = mybir.dt.float8e4
AF = mybir.ActivationFunctionType
OP = mybir.AluOpType
DR = mybir.MatmulPerfMode.DoubleRow


def _wcols(w):
    """[din, dout] dram AP -> [p, ko, dout] (feature-chunked lhsT view)."""
    return w.rearrange("(ko p) f -> p ko f", p=P)


def build_nc(taps=()):
    nc = bacc.Bacc("TRN2", target_bir_lowering=False, debug=False)

    d = {}
    d['xt'] = nc.dram_tensor("xt", [D, N], BF16, kind="ExternalInput").ap()
    d['ctx8'] = nc.dram_tensor("ctx8", [CD, S], F8, kind="ExternalInput").ap()
    d['cT'] = nc.dram_tensor("cT", [P, KD], F32, kind="ExternalInput").ap()
    for nm, sh in [("w_qkv8", [D, 3 * D]), ("w_so8", [D, D]), ("w_cq8", [D, D]),
                   ("w_ck8", [CD, D]), ("w_cv8", [CD, D]), ("w_co8", [D, D])]:
        d[nm] = nc.dram_tensor(nm, sh, F8, kind="ExternalInput").ap()
    for nm, sh in [("w1", [D, MLP]), ("w2", [MLP, D]), ("w_ada", [D, 6 * D])]:
        d[nm] = nc.dram_tensor(nm, sh, BF16, kind="ExternalInput").ap()
    for nm, w in [("bqkT", 16), ("bsoT", KD), ("bcqT", KD), ("bckT", KD),
                  ("bcoT", KD), ("b1T", MT), ("b2T", KD), ("badaT", 48)]:
        d[nm] = nc.dram_tensor(nm, [P, w], F32, kind="ExternalInput").ap()
    d['bv32'] = nc.dram_tensor("bv32", [D], F32, kind="ExternalInput").ap()
    d['bcv32'] = nc.dram_tensor("bcv32", [D], F32, kind="ExternalInput").ap()
    out = nc.dram_tensor("out_x", [D, N], F32, kind="ExternalOutput").ap()
    srows = nc.dram_tensor("srows", [32, N], F32).ap()

    tap_shapes = {
        "ada": ([P, 48], F32), "h1": ([P, KD, N], F8),
        "q": ([P, KD, N], BF16), "k": ([P, KD, N], BF16),
        "v65": ([P, NT, H, 65], BF16), "saO": ([P, KD, N], F8),
        "x2": ([P, KD, N], BF16), "h2": ([P, KD, N], F8),
        "cq": ([P, KD, N], BF16), "ck": ([P, KD, S], BF16),
        "cv65": ([P, ST, H, 65], BF16), "caO": ([P, KD, N], F8),
        "x3": ([P, KD, N], BF16), "h3": ([P, KD, N], BF16),
        "xT": ([P, KD, N], BF16), "g": ([P, MT, N], BF16),
    }
    tap_aps = {nm: nc.dram_tensor(f"dbg_{nm}", *tap_shapes[nm], kind="ExternalOutput").ap()
               for nm in taps}

    with tile.TileContext(nc) as tc:
        _emit(nc, tc, d, out, srows, tap_aps)
    nc.compile()
    return nc


def _emit(nc, tc, d, out, srows, tap_aps={}):
    def tap(nm, t):
        if nm in tap_aps:
            nc.sync.dma_start(tap_aps[nm], t[:])

    _ctr = [0]

    def nm(base):
        _ctr[0] += 1
        return f"{base}_{_ctr[0]}"

    gl = contextlib.ExitStack()
    with gl:
        const = gl.enter_context(tc.tile_pool(name="const", bufs=1))
        resid = gl.enter_context(tc.tile_pool(name="resid", bufs=2))
        actbf = gl.enter_context(tc.tile_pool(name="actbf", bufs=2))
        act8 = gl.enter_context(tc.tile_pool(name="act8", bufs=1))

        # ---------- constants ----------
        ident = const.tile([P, P], F32, tag="ident")
        make_identity(nc, ident)
        onesD_mat = const.tile([P, P], BF16, tag="onesD_mat")
        nc.vector.memset(onesD_mat[:], 1.0 / D)
        eps_t = const.tile([P, 1], F32, tag="eps")
        nc.vector.memset(eps_t[:], EPS)

        # pre-transposed biases straight from DRAM
        bt = {}
        for bn, w in [("bqkT", 16), ("bsoT", KD), ("bcqT", KD), ("bckT", KD),
                      ("bcoT", KD), ("b1T", MT), ("b2T", KD), ("badaT", 48)]:
            bt[bn] = const.tile([P, w], F32, tag=bn, name=bn)
            nc.sync.dma_start(bt[bn][:], d[bn])
        # x (feature-major, bf16) -- per-chunk DMAs so LN1 stats can start early
        xT = resid.tile([P, KD, N], BF16, tag="resid", name="xT")
        xt_cols = d['xt'].rearrange("(ko p) n -> p ko n", p=P)
        for k in range(KD):
            nc.sync.dma_start(xT[:, k, :], xt_cols[:, k, :])

        # silu(c) feature-major columns
        ctile = const.tile([P, KD], F32, tag="ctile")
        nc.sync.dma_start(ctile[:], d['cT'])
        silu_cT = const.tile([P, KD], BF16, tag="silu_cT")
        nc.scalar.activation(silu_cT[:], ctile[:], AF.Silu)

        ada = const.tile([P, 48], F32, tag="ada")
        splus = const.tile([P, 24], F32, tag="splus")

        # ---------- LayerNorm helpers ----------
        def ln_stats(x_in, stack):
            lnb = stack.enter_context(tc.tile_pool(name="lnb", bufs=3))
            ps_ln = stack.enter_context(tc.tile_pool(name="ps_ln", bufs=1, space="PSUM"))
            mu_ps = ps_ln.tile([P, N], F32, tag="mups")
            e2_ps = ps_ln.tile([P, N], F32, tag="e2ps")
            for k in range(KD):
                sq = lnb.tile([P, N], BF16, tag="lnsq", name=nm("lnsq"))
                nc.scalar.activation(sq[:], x_in[:, k], AF.Square)
                for half in range(2):
                    hs = slice(half * 512, (half + 1) * 512)
                    nc.tensor.matmul(mu_ps[:, hs], onesD_mat[:], x_in[:, k, hs],
                                     start=(k == 0), stop=(k == KD - 1))
                    nc.tensor.matmul(e2_ps[:, hs], onesD_mat[:], sq[:, hs],
                                     start=(k == 0), stop=(k == KD - 1))
            mu_bf = lnb.tile([P, N], BF16, tag="mubf")
            nc.vector.tensor_copy(mu_bf[:], mu_ps[:])
            rstd = lnb.tile([P, N], F32, tag="rstd")
            nc.vector.tensor_mul(rstd[:], mu_bf[:], mu_bf[:])
            nc.vector.tensor_sub(rstd[:], e2_ps[:], rstd[:])
            nc.scalar.activation(rstd[:], rstd[:], AF.Sqrt, bias=eps_t[:])
            nc.vector.reciprocal(rstd[:], rstd[:])
            rstd_bf = lnb.tile([P, N], BF16, tag="rstdbf")
            nc.vector.tensor_copy(rstd_bf[:], rstd[:])
            return mu_bf, rstd_bf

        def ln_apply(x_in, g, mu_bf, rstd_bf, h_out, tpool):
            for k in range(KD):
                t1 = tpool.tile([P, N], BF16, tag="t1", name=nm("t1"))
                nc.vector.tensor_sub(t1[:], x_in[:, k], mu_bf[:])
                nc.vector.tensor_mul(t1[:], t1[:], rstd_bf[:])
                nc.gpsimd.tensor_scalar(h_out[:, k], t1[:],
                                        splus[:, g * 8 + k:g * 8 + k + 1],
                                        ada[:, g * 16 + k:g * 16 + k + 1],
                                        OP.mult, OP.add)

        def ln_mod(x_in, g, h_out):
            ls = contextlib.ExitStack()
            with ls:
                mu_ps, rstd = ln_stats(x_in, ls)
                tp = ls.enter_context(tc.tile_pool(name="lnt", bufs=2))
                ln_apply(x_in, g, mu_ps, rstd, h_out, tp)

        # ---------- fp8 DoubleRow projections ----------
        def proj8(ps_pool, wp, w8cols, kdin, a8, n_free, dout, evict):
            nhalf = max(1, n_free // 512)
            nw = min(512, n_free)
            kp_n = kdin // 2
            for blk in range(dout // 1024):
                wb = wp.tile([P, kdin, 1024], F8, tag="w8", name=nm("w8"))
                nc.sync.dma_start(wb[:], w8cols[:, :, blk * 1024:(blk + 1) * 1024])
                for t8 in range(8):
                    pss = [ps_pool.tile([P, 512], F32, tag="pmm", name=nm("pmm")) for _ in range(nhalf)]
                    for kp in range(kp_n):
                        for half in range(nhalf):
                            hs = slice(half * 512, half * 512 + nw)
                            nc.tensor.matmul(pss[half][:, 0:nw],
                                             wb[:, 2 * kp:2 * kp + 2, t8 * 128:(t8 + 1) * 128],
                                             a8[:, 2 * kp:2 * kp + 2, hs],
                                             start=(kp == 0), stop=(kp == kp_n - 1),
                                             perf_mode=DR)
                    for half in range(nhalf):
                        evict(blk * 8 + t8, half, pss[half])

        def proj_V8(ps_pool, wp, w8cols, kdin, a8, m_tiles, v65t, bias_b):
            kp_n = kdin // 2
            wb = wp.tile([P, kdin, 1024], F8, tag="w8", name=nm("w8v"))
            nc.sync.dma_start(wb[:], w8cols[:])
            for i in range(m_tiles):
                ps0 = ps_pool.tile([P, 512], F32, tag="pmm", name=nm("pv0"))
                ps1 = ps_pool.tile([P, 512], F32, tag="pmm", name=nm("pv1"))
                for kp in range(kp_n):
                    a_sl = a8[:, 2 * kp:2 * kp + 2, i * 128:(i + 1) * 128]
                    nc.tensor.matmul(ps0[:], a_sl, wb[:, 2 * kp:2 * kp + 2, 0:512],
                                     start=(kp == 0), stop=(kp == kp_n - 1), perf_mode=DR)
                    nc.tensor.matmul(ps1[:], a_sl, wb[:, 2 * kp:2 * kp + 2, 512:1024],
                                     start=(kp == 0), stop=(kp == kp_n - 1), perf_mode=DR)
                for blk, ps in ((0, ps0), (1, ps1)):
                    nc.vector.tensor_add(
                        v65t[:, i, blk * 8:(blk + 1) * 8, 0:64],
                        ps.rearrange("p (h e) -> p h e", h=8),
                        bias_b[:, blk * 512:(blk + 1) * 512]
                        .rearrange("p (h e) -> p h e", h=8))
            nc.vector.memset(v65t[:, :, :, 64:65], WS)

        # ---------- attention core (bf16, head pairs) ----------
        def attention(q_T, kv_T, v65t, m_tiles, o8, srow_base):
            at = contextlib.ExitStack()
            with at:
                expp = at.enter_context(tc.tile_pool(name="expp", bufs=8))
                arows = at.enter_context(tc.tile_pool(name="arows", bufs=2))
                rb = at.enter_context(tc.tile_pool(name="rb", bufs=2))
                ps_lg = at.enter_context(tc.tile_pool(name="ps_lg", bufs=4, space="PSUM"))
                ps_pv = at.enter_context(tc.tile_pool(name="ps_pv", bufs=2, space="PSUM"))
                for hp in range(8):
                    ha, hb = 2 * hp, 2 * hp + 1
                    pv_a = ps_pv.tile([65, N], F32, tag="pv", name=f"pva{srow_base}_{hp}")
                    pv_b = ps_pv.tile([65, N], F32, tag="pv", name=f"pvb{srow_base}_{hp}")
                    items = m_tiles * 2
                    exs = [None] * items

                    def lgexp(i):
                        mt, half = i // 2, i % 2
                        hs = slice(half * 512, (half + 1) * 512)
                        pr = []
                        for idx, off in ((0, 0), (1, 64)):
                            lg = ps_lg.tile([P, 512], F32, tag="lg",
                                            name=f"lg{srow_base}_{hp}_{i}_{idx}")
                            nc.tensor.matmul(lg[:],
                                             kv_T[off:off + 64, hp, mt * 128:(mt + 1) * 128],
                                             q_T[off:off + 64, hp, hs],
                                             start=True, stop=True)
                            ex = expp.tile([P, 512], BF16, tag="ex",
                                           name=f"ex{srow_base}_{hp}_{i}_{idx}")
                            nc.scalar.activation(ex[:], lg[:], AF.Exp, scale=ASCALE)
                            pr.append(ex)
                        exs[i] = pr

                    def pvacc(i):
                        mt, half = i // 2, i % 2
                        hs = slice(half * 512, (half + 1) * 512)
                        nc.tensor.matmul(pv_a[:, hs], v65t[:, mt, ha, :], exs[i][0][:],
                                         start=(mt == 0), stop=(mt == m_tiles - 1))
                        nc.tensor.matmul(pv_b[:, hs], v65t[:, mt, hb, :], exs[i][1][:],
                                         start=(mt == 0), stop=(mt == m_tiles - 1))

                    for i in range(items + 2):
                        if i < items:
                            lgexp(i)
                        if i >= 2:
                            pvacc(i - 2)
                    for idx, (pv, h) in enumerate(((pv_a, ha), (pv_b, hb))):
                        # copy PSUM out right away so the bank pair frees for
                        # the next head pair; normalize from SBUF
                        pvs = rb.tile([65, N], F32, tag="pvs", name=nm("pvs"))
                        nc.vector.tensor_copy(pvs[:], pv[:])
                        rec = arows.tile([1, N], F32, tag="row", name=nm("rec"))
                        nc.vector.reciprocal(rec[:], pvs[64:65, :])
                        r = srow_base + h
                        nc.sync.dma_start(srows[r:r + 1, :], rec[:])
                        rbt = rb.tile([64, N], F32, tag="rbt", name=nm("rbt"))
                        nc.sync.dma_start(rbt[:], srows[r:r + 1, :].partition_broadcast(64))
                        off = idx * 64
                        nc.vector.tensor_mul(o8[off:off + 64, hp, :], pvs[0:64, :], rbt[:])

        # ================= phase 0: LN1 stats + ada =================
        ln1 = contextlib.ExitStack()
        mu1, rstd1 = ln_stats(xT, ln1)

        ada_es = contextlib.ExitStack()
        with ada_es:
            adap = ada_es.enter_context(tc.tile_pool(name="adap", bufs=2))
            wadap = ada_es.enter_context(tc.tile_pool(name="wadap", bufs=2))
            ps_ada = ada_es.enter_context(tc.tile_pool(name="ps_ada", bufs=2, space="PSUM"))
            ps_tr = ada_es.enter_context(tc.tile_pool(name="ps_tr", bufs=1, space="PSUM"))
            adarow = adap.tile([12, 512], F32, tag="adarow")
            wada_cols = _wcols(d['w_ada'])
            for blk in range(6):
                wb = wadap.tile([P, KD, 1024], BF16, tag="wada", name=nm("wada"))
                nc.sync.dma_start(wb[:], wada_cols[:, :, blk * 1024:(blk + 1) * 1024])
                for tb in range(2):
                    ps = ps_ada.tile([1, 512], F32, tag="psada", name=nm("psada"))
                    for k in range(KD):
                        nc.tensor.matmul(ps[:], silu_cT[:, k:k + 1],
                                         wb[:, k, tb * 512:(tb + 1) * 512],
                                         start=(k == 0), stop=(k == KD - 1))
                    r = blk * 2 + tb
                    ast = adap.tile([1, 512], F32, tag="ast", name=nm("ast"))
                    nc.scalar.activation(ast[:], ps[:], AF.Copy)
                    nc.sync.dma_start(adarow[r:r + 1, :], ast[:])
            # transpose adarow [12,512] -> ada [P,48]
            tp = ps_tr.tile([P, 512], F32, tag="ptr")
            for j in range(4):
                nc.tensor.transpose(tp[:, j * 12:(j + 1) * 12],
                                    adarow[:, j * 128:(j + 1) * 128], ident[0:12, 0:12])
            ada4 = ada.rearrange("p (r j) -> p r j", j=4)
            for j in range(4):
                nc.vector.tensor_copy(ada4[:, :, j], tp[:, j * 12:(j + 1) * 12])
            nc.vector.tensor_add(ada[:], ada[:], bt['badaT'][:])
            for g in range(3):
                nc.vector.tensor_scalar_add(splus[:, g * 8:(g + 1) * 8],
                                            ada[:, g * 16 + 8:g * 16 + 16], 1.0)
        tap("ada", ada)
        tap("xT", xT)

        # deferred non-critical loads (keep HBM clear for w_ada up front)
        ctx8 = const.tile([P, CKD, S], F8, tag="ctx8")
        nc.sync.dma_start(ctx8[:], d['ctx8'].rearrange("(ko p) n -> p ko n", p=P))
        vbias = const.tile([P, D], F32, tag="vbias")
        nc.sync.dma_start(vbias[:], d['bv32'][None, :].partition_broadcast(P))
        cvbias = const.tile([P, D], F32, tag="cvbias")
        nc.sync.dma_start(cvbias[:], d['bcv32'][None, :].partition_broadcast(P))

        # ================= LN1 apply -> h1 (fp8) =================
        h1 = act8.tile([P, KD, N], F8, tag="a8", name="h1")
        lnt1 = contextlib.ExitStack()
        with lnt1:
            tp1 = lnt1.enter_context(tc.tile_pool(name="lnt", bufs=2))
            ln_apply(xT, 0, mu1, rstd1, h1, tp1)
        ln1.close()
        tap("h1", h1)

        # fold so-bias into the residual (xT last read by ln_apply above)
        for k in range(KD):
            nc.gpsimd.tensor_scalar_add(xT[:, k, :], xT[:, k, :], bt['bsoT'][:, k:k + 1])

        # ================= self-attention =================
        sa_es = contextlib.ExitStack()
        vp = sa_es.enter_context(tc.tile_pool(name="vp", bufs=1))
        v65 = vp.tile([P, NT, H, 65], BF16, tag="v65")

        qkv_ps = contextlib.ExitStack()
        ps_mm = qkv_ps.enter_context(tc.tile_pool(name="ps_mm", bufs=4, space="PSUM"))
        wq_p = qkv_ps.enter_context(tc.tile_pool(name="wq_p", bufs=2))
        wq_cols = _wcols(d['w_qkv8'])
        proj_V8(ps_mm, wq_p, wq_cols[:, :, 2 * D:3 * D], KD, h1, NT, v65, vbias)

        qT = actbf.tile([P, KD, N], BF16, tag="abf", name="qT")
        kT = actbf.tile([P, KD, N], BF16, tag="abf", name="kT")

        def ev_qk(t, half, ps):
            dst = qT if t < 8 else kT
            hs = slice(half * 512, (half + 1) * 512)
            nc.scalar.activation(dst[:, t % 8, hs], ps[:], AF.Identity,
                                 bias=bt['bqkT'][:, t:t + 1], scale=IWS)
        proj8(ps_mm, wq_p, wq_cols[:, :, 0:2 * D], KD, h1, N, 2 * D, ev_qk)
        tap("q", qT); tap("k", kT); tap("v65", v65)
        qkv_ps.close()

        saO = act8.tile([P, KD, N], F8, tag="a8", name="saO")
        attention(qT, kT, v65, NT, saO, 0)
        tap("saO", saO)
        sa_es.close()

        x2T = resid.tile([P, KD, N], BF16, tag="resid", name="x2T")
        so_ps = contextlib.ExitStack()
        ps_mm = so_ps.enter_context(tc.tile_pool(name="ps_mm", bufs=4, space="PSUM"))
        wso_p = so_ps.enter_context(tc.tile_pool(name="wso_p", bufs=1))

        def ev_so(t, half, ps):
            hs = slice(half * 512, (half + 1) * 512)
            nc.vector.scalar_tensor_tensor(x2T[:, t, hs], ps[:], IWS, xT[:, t, hs],
                                           OP.mult, OP.add)
        proj8(ps_mm, wso_p, _wcols(d['w_so8']), KD, saO, N, D, ev_so)
        tap("x2", x2T)
        so_ps.close()

        # ================= cross-attention =================
        ca_es = contextlib.ExitStack()
        kp_ = ca_es.enter_context(tc.tile_pool(name="kp", bufs=1))
        vp2 = ca_es.enter_context(tc.tile_pool(name="vp2", bufs=1))
        ckT = kp_.tile([P, KD, S], BF16, tag="ckT")
        cv65 = vp2.tile([P, ST, H, 65], BF16, tag="cv65")

        ckcv_ps = contextlib.ExitStack()
        ps_kv = ckcv_ps.enter_context(tc.tile_pool(name="ps_kv", bufs=2, space="PSUM"))
        wkv_p = ckcv_ps.enter_context(tc.tile_pool(name="wkv_p", bufs=2))

        def ev_ck(t, half, ps):
            nc.scalar.activation(ckT[:, t, :], ps[:, 0:S], AF.Identity,
                                 bias=bt['bckT'][:, t:t + 1], scale=IWS)
        proj8(ps_kv, wkv_p, _wcols(d['w_ck8']), CKD, ctx8, S, D, ev_ck)
        proj_V8(ps_kv, wkv_p, _wcols(d['w_cv8']), CKD, ctx8, ST, cv65, cvbias)
        tap("ck", ckT); tap("cv65", cv65)

        h2 = act8.tile([P, KD, N], F8, tag="a8", name="h2")
        ln_mod(x2T, 1, h2)
        ckcv_ps.close()
        tap("h2", h2)

        # fold co-bias into x2T (last read by ln_mod above)
        for k in range(KD):
            nc.gpsimd.tensor_scalar_add(x2T[:, k, :], x2T[:, k, :], bt['bcoT'][:, k:k + 1])

        cqT = actbf.tile([P, KD, N], BF16, tag="abf", name="cqT")
        ca_ps = contextlib.ExitStack()
        ps_mm = ca_ps.enter_context(tc.tile_pool(name="ps_mm", bufs=4, space="PSUM"))
        wcq_p = ca_ps.enter_context(tc.tile_pool(name="wcq_p", bufs=1))

        def ev_cq(t, half, ps):
            hs = slice(half * 512, (half + 1) * 512)
            nc.scalar.activation(cqT[:, t, hs], ps[:], AF.Identity,
                                 bias=bt['bcqT'][:, t:t + 1], scale=IWS)
        proj8(ps_mm, wcq_p, _wcols(d['w_cq8']), KD, h2, N, D, ev_cq)
        tap("cq", cqT)
        ca_ps.close()

        caO = act8.tile([P, KD, N], F8, tag="a8", name="caO")
        attention(cqT, ckT, cv65, ST, caO, 16)
        tap("caO", caO)
        ca_es.close()

        x3T = resid.tile([P, KD, N], BF16, tag="resid", name="x3T")
        co_ps = contextlib.ExitStack()
        ps_mm = co_ps.enter_context(tc.tile_pool(name="ps_mm", bufs=4, space="PSUM"))
        wco_p = co_ps.enter_context(tc.tile_pool(name="wco_p", bufs=1))

        def ev_co(t, half, ps):
            hs = slice(half * 512, (half + 1) * 512)
            nc.vector.scalar_tensor_tensor(x3T[:, t, hs], ps[:], IWS, x2T[:, t, hs],
                                           OP.mult, OP.add)
        proj8(ps_mm, wco_p, _wcols(d['w_co8']), KD, caO, N, D, ev_co)
        tap("x3", x3T)
        co_ps.close()

        # ================= FFN =================
        h3 = actbf.tile([P, KD, N], BF16, tag="abf", name="h3")
        ln_mod(x3T, 2, h3)
        tap("h3", h3)
        # fold b2 into the residual before the final eviction
        for k in range(KD):
            nc.gpsimd.tensor_scalar_add(x3T[:, k, :], x3T[:, k, :], bt['b2T'][:, k:k + 1])

        ffn_es = contextlib.ExitStack()
        gp = ffn_es.enter_context(tc.tile_pool(name="gp", bufs=1))
        g = gp.tile([P, MT, N], BF16, tag="g")

        w1_es = contextlib.ExitStack()
        ps_mm = w1_es.enter_context(tc.tile_pool(name="ps_mm", bufs=4, space="PSUM"))
        w1_p = w1_es.enter_context(tc.tile_pool(name="w1_p", bufs=2))
        w1_cols = _wcols(d['w1'])
        for blk in range(4):
            wb = w1_p.tile([P, KD, 1024], BF16, tag="w16", name=nm("w16"))
            nc.sync.dma_start(wb[:], w1_cols[:, :, blk * 1024:(blk + 1) * 1024])
            for t8 in range(8):
                t = blk * 8 + t8
                ps0 = ps_mm.tile([P, 512], F32, tag="pmm", name=nm("pw0"))
                ps1 = ps_mm.tile([P, 512], F32, tag="pmm", name=nm("pw1"))
                for k in range(KD):
                    nc.tensor.matmul(ps0[:], wb[:, k, t8 * 128:(t8 + 1) * 128],
                                     h3[:, k, 0:512], start=(k == 0), stop=(k == KD - 1))
                    nc.tensor.matmul(ps1[:], wb[:, k, t8 * 128:(t8 + 1) * 128],
                                     h3[:, k, 512:1024], start=(k == 0), stop=(k == KD - 1))
                nc.scalar.activation(g[:, t, 0:512], ps0[:], AF.Gelu,
                                     bias=bt['b1T'][:, t:t + 1])
                nc.scalar.activation(g[:, t, 512:1024], ps1[:], AF.Gelu,
                                     bias=bt['b1T'][:, t:t + 1])
        tap("g", g)
        w1_es.close()

        w2_es = contextlib.ExitStack()
        outst = w2_es.enter_context(tc.tile_pool(name="outst", bufs=4))
        ps_po = w2_es.enter_context(tc.tile_pool(name="ps_po", bufs=4, space="PSUM"))
        w2_p = w2_es.enter_context(tc.tile_pool(name="w2_p", bufs=2))
        w2_cols = d['w2'].rearrange("(ko p) f -> p ko f", p=P)
        for t8 in range(8):
            wb = w2_p.tile([P, MT, P], BF16, tag="w2b", name=nm("w2b"))
            nc.sync.dma_start(wb[:], w2_cols[:, :, t8 * 128:(t8 + 1) * 128])
            ps0 = ps_po.tile([P, 512], F32, tag="po", name=nm("po0"))
            ps1 = ps_po.tile([P, 512], F32, tag="po", name=nm("po1"))
            for m in range(MT):
                nc.tensor.matmul(ps0[:], wb[:, m, :], g[:, m, 0:512],
                                 start=(m == 0), stop=(m == MT - 1))
                nc.tensor.matmul(ps1[:], wb[:, m, :], g[:, m, 512:1024],
                                 start=(m == 0), stop=(m == MT - 1))
            for half, ps in ((0, ps0), (1, ps1)):
                hs = slice(half * 512, (half + 1) * 512)
                ost = outst.tile([P, 512], F32, tag="ost", name=nm("ost"))
                nc.vector.tensor_add(ost[:], ps[:], x3T[:, t8, hs])
                nc.sync.dma_start(out[t8 * 128:(t8 + 1) * 128, hs], ost[:])
        w2_es.close()
        ffn_es.close()


_NC = None


def _get_nc():
    global _NC
    if _NC is None:
        _NC = build_nc()
    return _NC


def make_in_maps(inputs):
    f8 = ml_dtypes.float8_e4m3
    bf = ml_dtypes.bfloat16
    f32 = np.float32
    shared = {}
    for src, dst in [("w_qkv", "w_qkv8"), ("w_so", "w_so8"), ("w_cq", "w_cq8"),
                     ("w_ck", "w_ck8"), ("w_cv", "w_cv8"), ("w_co", "w_co8")]:
        shared[dst] = np.ascontiguousarray(
            (np.asarray(inputs[src], f32) * WS).astype(f8))
    for nm in ("w1", "w2", "w_ada"):
        shared[nm] = np.ascontiguousarray(np.asarray(inputs[nm]).astype(bf))
    bq = np.asarray(inputs['b_qkv'], f32)
    shared['bqkT'] = np.ascontiguousarray(bq[:2 * D].reshape(16, P).T)
    shared['bv32'] = np.ascontiguousarray(WS * bq[2 * D:])
    shared['bcv32'] = np.ascontiguousarray(WS * np.asarray(inputs['b_cv'], f32))
    for src, dst, w in [("b_so", "bsoT", KD), ("b_cq", "bcqT", KD),
                        ("b_ck", "bckT", KD), ("b_co", "bcoT", KD),
                        ("b1", "b1T", MT), ("b2", "b2T", KD),
                        ("b_ada", "badaT", 48)]:
        shared[dst] = np.ascontiguousarray(
            np.asarray(inputs[src], f32).reshape(w, P).T)
    x = np.asarray(inputs['x'], f32)
    c = np.asarray(inputs['c'], f32)
    ctxt = np.asarray(inputs['context'], f32)
    in_maps = []
    for i in range(NCORES):
        m = dict(shared)
        m['xt'] = np.ascontiguousarray(x[i].T.astype(bf))
        m['ctx8'] = np.ascontiguousarray(ctxt[i].T.astype(f8))
        m['cT'] = np.ascontiguousarray(c[i].reshape(KD, P).T)
        in_maps.append(m)
    return in_maps


def kernel(**inputs):
    nc = _get_nc()
    in_maps = make_in_maps(inputs)
    res = run_bass_kernel_spmd(nc, in_maps, core_ids=list(range(NCORES)))
    return np.stack([res.results[i]["out_x"].T for i in range(NCORES)]).astype(np.float32)


if __name__ == "__main__":
    data = np.load("/root/problem/inputs.npz")
    out = kernel(**{k: data[k] for k in data.files})
    gold = np.load("/root/problem/gold64.npy")
    err = np.abs(out - gold)
    print("max abs err:", err.max(), " rel:", err.max() / np.abs(gold).max())
